# revision 15
# baseline (speedup 1.0000x reference)
"""CrossNet layer (encoder Dense + 4 cross layers) on 8 trn2 NeuronCores.

Pure data parallelism: batch 1024 is split into 8 shards of 128 rows;
encoder weights + tiny cross weights are replicated per core.

Math: with h = x @ W_enc + b_enc, x0 = h, the cross recurrence
    x_{l+1} = x_l + x0 * (x_l @ w_l) + b_l
keeps the closed form x_l = x0 * c_l + B_l with per-row scalar c_l and
H-vector B_l = sum_{j<l} b_j, since
    s_l = x_l @ w_l = c_l * (x0 @ w_l) + B_l @ w_l = c_l * p_l + q_l
    c_{l+1} = c_l * (1 + p_l) + q_l,   c_0 = 1.
So the device needs the big matmul h, P = x0 @ Wc (Wc = ws^T), the 4x4
table Q[j,l] = b_j @ w_l (q_l = sum_{j<l} Q[j,l]), a 4-step scan for c,
and out = x0 * c_4 + B_4.

All tensors run bf16 (host-rounded; rel err ~3e-3 vs the 2e-2 gate):
half the HBM bytes and full-rate PE (measured ~216ns throughput per
[128x128]@[128x512] matmul back-to-back vs 774ns for f32r). The cross
weights are folded host-side into Wc' = W_enc @ ws^T (16KB, standard
weight fusion), so P = x @ Wc' accumulates directly off the x^T
stationary tiles inside the main k-loop - no h transposes, no tail
pipeline. The final out = h*c4 + B4 reads h straight from PSUM.

Layouts are packed host-side so every DMA is contiguous with max-size
elements. The whole critical stream (x^T then the 4 W chunks) rides the
fast SP HWDGE ring in consumption order - per-queue FIFO keeps arrivals
ordered at ~340GB/s, while the ACT ring (measured ~25-45GB/s) only
carries the tiny ws/bs/be packs. No scalar.copy anywhere, so the 1.3us
one-time ACT table program (PWP) never happens. The framework preamble's
dead weight (bounds-check reg inits, no-reader const memsets) is
trimmed from the module before codegen.
"""

import numpy as np
import ml_dtypes

B, D, H, DEPTH = 1024, 1024, 1024, 4
N_CORES = 8
BS = B // N_CORES   # batch rows per core
KT = D // 128       # contraction k-tiles
NT = H // 512       # psum n-tiles
NCHUNK = 4          # W dma chunks
KPC = KT // NCHUNK  # k-tiles per chunk

BF16 = ml_dtypes.bfloat16
WBP_COLS = 256  # wst/bst pack | Wc' pack | bewc row (padded to 512B lines)

_cache = {}


def _patch_tile_drain(max_waits: int = 1):
    """walrus in this image allows only 1 sync-wait per instruction; the stock
    Tile end-of-kernel drain carries the whole global clock on one SP Drain and
    codegen fails. Split the waits across a chain of SP nops instead."""
    import concourse.tile as tile
    from concourse.vector_clock import ScopedClock
    from concourse import mybir

    if getattr(tile.TileContext, "_drain_patched", False):
        return

    def _drain_and_barrier(self, tick_clock, wait_clock):
        nc = self.nc
        carrier = nc.sync.nop()
        wait_clock.add_sem_waits(
            carrier.ins, ScopedClock({None: tick_clock.global_clock})
        )
        si = carrier.ins.sync_info
        if si is not None and si.on_wait and len(si.on_wait) > max_waits:
            waits = list(si.on_wait)
            carrier.ins.sync_info = mybir.SyncInfo(
                on_wait=waits[:max_waits], on_update=list(si.on_update or [])
            )
            rest = waits[max_waits:]
            while rest:
                extra = nc.sync.nop()
                extra.ins.sync_info = mybir.SyncInfo(
                    on_wait=rest[:max_waits], on_update=[]
                )
                rest = rest[max_waits:]
        nc.sync.drain()

        # exit barrier + sem clears dropped: the NEFF preamble re-inits
        # semaphores on every execution (verified by back-to-back runs), so
        # the ~4us exit butterfly only burns measured time
        assert self.sems is not None
        popped = nc._tile_sem_poison_stack.pop()
        assert popped is self._sem_poison

    tile.TileContext._drain_and_barrier = _drain_and_barrier
    tile.TileContext._drain_patched = True


def _split_multi_waits(nc):
    """walrus here allows only one sync-wait per instruction: move extra waits
    onto same-engine NoOps inserted immediately before the instruction."""
    from concourse import mybir

    for fn in nc.m.functions:
        for bb in fn.blocks:
            out = []
            for inst in bb.instructions:
                si = inst.sync_info
                if si is not None and si.on_wait and len(si.on_wait) > 1:
                    waits = list(si.on_wait)
                    for i, w in enumerate(waits[:-1]):
                        nop = mybir.InstNoOp(name=f"{inst.name}-w{i}", ins=[], outs=[])
                        nop.engine = inst.engine
                        nop.sync_info = mybir.SyncInfo(on_wait=[w], on_update=[])
                        out.append(nop)
                    inst.sync_info = mybir.SyncInfo(
                        on_wait=[waits[-1]], on_update=list(si.on_update or [])
                    )
                out.append(inst)
            bb.instructions[:] = out


def _trim_preamble(nc):
    """Delete the framework preamble's dead weight: per-engine bounds-check
    register inits (no dynamic APs in this kernel -> no readers) and the
    4 framework const memsets (verifier reports no readers). They run
    before the entry barrier, so this pulls the whole body ~1us earlier."""
    fn = nc.m.functions[0]
    bb = fn.blocks[0]
    drop = ("InstRegisterMove", "InstMemset")
    bb.instructions[:] = [
        i for i in bb.instructions if type(i).__name__ not in drop
    ]


def _build(split=True, trim=True):
    from contextlib import ExitStack

    import concourse.bass as bass
    import concourse.tile as tile
    from concourse import mybir

    _patch_tile_drain()

    fp32 = mybir.dt.float32
    bf16 = mybir.dt.bfloat16
    i32 = mybir.dt.int32
    Alu = mybir.AluOpType

    nc = bass.Bass()
    xt_in = nc.declare_dram_parameter("xt", [128, KT * BS], bf16, isOutput=False)
    wc_in = [
        nc.declare_dram_parameter(f"wc{c}", [128, KPC * H], bf16, isOutput=False)
        for c in range(NCHUNK)
    ]
    wbp_in = nc.declare_dram_parameter("wbp", [128, WBP_COLS], bf16, isOutput=False)
    bsr_in = nc.declare_dram_parameter("bsr", [DEPTH, H], bf16, isOutput=False)
    ber_in = nc.declare_dram_parameter("ber", [1, H], bf16, isOutput=False)
    y_out = nc.declare_dram_parameter("y", [BS, H], bf16, isOutput=True)

    WCP0 = KT * 8       # wbp column offset of the Wc' pack
    BEWC0 = KT * 12     # wbp column offset of the bewc row

    with ExitStack() as ctx:
        tc = ctx.enter_context(tile.TileContext(nc))
        cpool = ctx.enter_context(tc.tile_pool(name="const", bufs=1))
        wpool = ctx.enter_context(tc.tile_pool(name="w", bufs=2 * NCHUNK))
        iop = ctx.enter_context(tc.tile_pool(name="io", bufs=1))
        psh = ctx.enter_context(tc.tile_pool(name="psh", bufs=2, space="PSUM"))
        psb = ctx.enter_context(tc.tile_pool(name="psb", bufs=2, space="PSUM"))
        psq = ctx.enter_context(tc.tile_pool(name="psq", bufs=1, space="PSUM"))

        # ---- input DMAs first ------------------------------------------
        # The whole critical stream rides ONE ring (SP) in consumption
        # order: queues service sub-dmas FIFO, so arrivals stay ordered
        # xt -> wc0..wc3 at full ring bandwidth (~330GB/s; chip-contention
        # bound with 8 cores). Parallel rings only scramble completion
        # order. Smalls ride the ACT ring so the Q/P warm-up matmuls
        # unblock early.
        xt_sb = iop.tile([128, KT * BS], bf16)
        nc.sync.dma_start(xt_sb[:], xt_in[:])
        w_sb = [
            wpool.tile([128, KPC * H], bf16, tag="wr", name=f"wr{c}")
            for c in range(NCHUNK)
        ]
        for c in range(NCHUNK):
            nc.sync.dma_start(w_sb[c][:], wc_in[c][:])
        wbp_sb = iop.tile([128, WBP_COLS], bf16)
        nc.scalar.dma_start(wbp_sb[:], wbp_in[:])
        bsr_sb = iop.tile([DEPTH, H], bf16)
        nc.scalar.dma_start(bsr_sb[:], bsr_in[:])
        ber_sb = iop.tile([1, H], bf16)
        nc.scalar.dma_start(ber_sb[:], ber_in[:])

        # ---- constants (overlap the DMA stream) -------------------------
        ones1 = cpool.tile([1, 128], fp32)
        nc.gpsimd.memset(ones1[:], 1.0)
        ones1b = cpool.tile([1, 128], bf16)
        nc.vector.tensor_copy(ones1b[:], ones1[:])
        ones4 = cpool.tile([4, 128], fp32)
        nc.gpsimd.memset(ones4[:], 1.0)
        ones4b = cpool.tile([4, 128], bf16)
        nc.vector.tensor_copy(ones4b[:], ones4[:])
        row4 = cpool.tile([4, 4], i32)
        col4 = cpool.tile([4, 4], i32)
        nc.gpsimd.iota(row4[:], pattern=[[0, 4]], base=0, channel_multiplier=1)
        nc.gpsimd.iota(col4[:], pattern=[[1, 4]], base=0, channel_multiplier=0)
        maskL = cpool.tile([4, 4], fp32)  # maskL[j,l] = 1 if j < l
        nc.vector.tensor_tensor(maskL[:], row4[:], col4[:], Alu.is_lt)

        # ---- Q = Bs^T.T @ Wc -> q_l = sum_{j<l} Q[j,l] ------------------
        q_ps = psq.tile([4, 4], fp32, tag="q")
        for k in range(KT):
            nc.tensor.matmul(
                q_ps[:], wbp_sb[:, k * 8 + 4 : k * 8 + 8], wbp_sb[:, k * 8 : k * 8 + 4],
                start=(k == 0), stop=(k == KT - 1),
            )
        qm_sb = cpool.tile([4, 4], bf16)
        nc.vector.tensor_tensor(qm_sb[:], q_ps[:], maskL[:], Alu.mult)
        qrow_ps = psq.tile([1, 4], fp32, tag="q")
        nc.tensor.matmul(qrow_ps[:], ones4b[:, 0:1], qm_sb[:], start=True, stop=True)
        qrow_sb = cpool.tile([1, 4], bf16)
        nc.vector.tensor_copy(qrow_sb[:], qrow_ps[:])
        qb_ps = psq.tile([128, 4], fp32, tag="q")
        nc.tensor.matmul(qb_ps[:], ones1b[:], qrow_sb[:], start=True, stop=True)

        # ---- big matmul h = x @ W + be, P = x @ Wc' + be@Wc -------------
        # k-outer; P accumulates in the same loop reusing the x^T tile
        # already in the stationary register. bias/B4 matmuls are emitted
        # after k1 so they fill the PE's wait for W chunk 1.
        h_ps = [psh.tile([128, 512], fp32, tag="hps", name=f"hps{n}") for n in range(NT)]
        p_ps = psq.tile([128, 4], fp32, tag="p")
        b4_ps = []

        def kmm(k, n, start=False, stop=False):
            c, kk = divmod(k, KPC)
            nc.tensor.matmul(
                h_ps[n][:],
                xt_sb[:, k * BS : (k + 1) * BS],
                w_sb[c][:, kk * H + n * 512 : kk * H + (n + 1) * 512],
                start=start, stop=stop, skip_group_check=True,
            )

        def pmm(k, start=False, stop=False):
            nc.tensor.matmul(
                p_ps[:],
                xt_sb[:, k * BS : (k + 1) * BS],
                wbp_sb[:, WCP0 + k * 4 : WCP0 + (k + 1) * 4],
                start=start, stop=stop, skip_group_check=True,
            )

        for k in (0, 1):
            kmm(k, 0, start=(k == 0))
            kmm(k, 1, start=(k == 0))
            pmm(k, start=(k == 0))
        # gap fillers while chunk 1 streams: h bias, B4 rows
        for n in range(NT):
            nc.tensor.matmul(
                h_ps[n][:], ones1b[:], ber_sb[:, n * 512 : (n + 1) * 512],
                start=False, stop=False, skip_group_check=True,
            )
        for n in range(NT):
            b4 = psb.tile([128, 512], fp32, tag="b4", name=f"b4ps{n}")
            nc.tensor.matmul(
                b4[:], ones4b[:], bsr_sb[:, n * 512 : (n + 1) * 512],
                start=True, stop=True,
            )
            b4_ps.append(b4)
        for k in range(2, KT - KPC):
            kmm(k, 0)
            kmm(k, 1)
            pmm(k)
        # last chunk: close the P group first (scan runs during the last
        # h matmuls), and finish half 0 before half 1 so its stt+DMA lead
        k0, k1 = KT - KPC, KT - 1
        pmm(k0)
        kmm(k0, 0)
        pmm(k1, stop=False)
        kmm(k1, 0, stop=True)
        nc.tensor.matmul(  # P bias: + be @ Wc
            p_ps[:], ones1b[:], wbp_sb[0:1, BEWC0 : BEWC0 + 4],
            start=False, stop=True, skip_group_check=True,
        )
        kmm(k0, 1)
        kmm(k1, 1, stop=True)

        # ---- b4 to SBUF early (frees stt to read h from PSUM) -----------
        b4_sb = iop.tile([BS, H], fp32)
        for n in range(NT):
            nc.vector.tensor_copy(b4_sb[:, n * 512 : (n + 1) * 512], b4_ps[n][:])

        # ---- c scan: c_{l+1} = (1 + P_l) * c_l + q_l --------------------
        at_sb = cpool.tile([128, 4], fp32)
        nc.vector.tensor_scalar_add(at_sb[:], p_ps[:], 1.0)
        c_sb = cpool.tile([128, 4], fp32)
        nc.vector.tensor_tensor_scan(
            c_sb[:], at_sb[:], qb_ps[:], 1.0, Alu.mult, Alu.add
        )

        # ---- final out = h * c4 + B4 straight from PSUM, per half -------
        out_sb = iop.tile([BS, H], bf16)
        # both halves on DVE: gpsimd cannot read PSUM on this HW. half 0's
        # stt runs while PE still does half 1's last matmuls.
        for n in range(NT):
            nc.vector.scalar_tensor_tensor(
                out_sb[:, n * 512 : (n + 1) * 512],
                h_ps[n][:],
                c_sb[:, 3:4],
                b4_sb[:, n * 512 : (n + 1) * 512],
                Alu.mult,
                Alu.add,
            )
            nc.scalar.dma_start(
                y_out[:, n * 512 : (n + 1) * 512], out_sb[:, n * 512 : (n + 1) * 512]
            )

    if trim:
        _trim_preamble(nc)
    if split:
        _split_multi_waits(nc)
    return nc


def _pack_inputs(x, W_enc, b_enc, ws, bs):
    """Host-side layout: shard x over cores, bf16-round, pre-transpose,
    and fold the tiny cross weights: Wc' = W_enc @ ws^T, bewc = be @ ws^T."""
    x = np.ascontiguousarray(x, dtype=np.float32)
    W = np.ascontiguousarray(W_enc, dtype=np.float32)
    ws2 = np.ascontiguousarray(ws, dtype=np.float32).reshape(DEPTH, H)
    bs2 = np.ascontiguousarray(bs, dtype=np.float32).reshape(DEPTH, H)
    be = np.ascontiguousarray(b_enc, dtype=np.float32).reshape(1, H)

    Wk = W.reshape(KT, 128, H)
    wcs = [
        np.ascontiguousarray(
            Wk[c * KPC : (c + 1) * KPC].transpose(1, 0, 2), dtype=BF16
        ).reshape(128, KPC * H)
        for c in range(NCHUNK)
    ]
    # wbp pack: [:, k*8:k*8+4]=ws^T tile, [.,+4:+8]=bs^T tile,
    # [:, KT*8 + k*4 : ...]=Wc' tile, [0, KT*12:+4]=be@Wc
    wst = ws2.reshape(DEPTH, KT, 128).transpose(2, 1, 0)  # [128, KT, 4]
    bst = bs2.reshape(DEPTH, KT, 128).transpose(2, 1, 0)
    wcp = (W @ ws2.T).reshape(KT, 128, DEPTH).transpose(1, 0, 2)  # [128, KT, 4]
    bewc = (be @ ws2.T).reshape(1, DEPTH)  # [1, 4]
    wbp = np.zeros((128, WBP_COLS), dtype=BF16)  # padded to 256 cols
    wbp[:, : KT * 8] = np.concatenate([wst, bst], axis=2).reshape(128, KT * 8)
    wbp[:, KT * 8 : KT * 12] = wcp.reshape(128, KT * 4)
    wbp[0, KT * 12 : KT * 12 + 4] = bewc[0]
    bsr = bs2.astype(BF16)
    ber = be.astype(BF16)

    in_maps = []
    for c in range(N_CORES):
        xs = x[c * BS : (c + 1) * BS]  # [BS, D]
        xt = np.ascontiguousarray(
            xs.T.reshape(KT, 128, BS).transpose(1, 0, 2), dtype=BF16
        ).reshape(128, KT * BS)
        m = {"xt": xt, "wbp": wbp, "bsr": bsr, "ber": ber}
        for ci in range(NCHUNK):
            m[f"wc{ci}"] = wcs[ci]
        in_maps.append(m)
    return in_maps


def kernel(x, W_enc, b_enc, ws, bs):
    from concourse.bass_utils import run_bass_kernel_spmd

    if "nc" not in _cache:
        _cache["nc"] = _build()
    nc = _cache["nc"]

    in_maps = _pack_inputs(x, W_enc, b_enc, ws, bs)
    res = run_bass_kernel_spmd(nc, in_maps, list(range(N_CORES)))
    return np.concatenate(
        [res.results[c]["y"].astype(np.float32) for c in range(N_CORES)], axis=0
    )
